# revision 43
# baseline (speedup 1.0000x reference)
"""BiAttention (BiDAF-style) kernel for Trainium2, 8 NeuronCores.

Reference math (T=4096, d=512):
    context  = x[0,0]; question = x[1,0]
    S[i,j]   = w1.c_i + w2.q_j + (c_i*w3).q_j
    A        = softmax_j(S)          # w1.c_i is constant per row -> cancels
    U_A      = A @ question
    b        = max_j A[i,j]          # == max_j E[i,j] / Z_i  with E=exp(S)
    h        = b @ context           # global over T -> one AllReduce
    G        = [context, U_A, context*U_A, context*h]

Sharding: context rows (and rows of S/A/U_A/G) split across 8 cores
(512 rows each); question replicated; h all-reduced (2 KB).

Per-core schedule:
  phase 1 (per 512-wide j-slab): SWDGE cast-load q slab (fp16), PE-transpose
    to qT, q2 row via matmul, S = cw3T.T @ qT + q2 (K=1 ones-fold matmul)
    for all four i-blocks, exp on ACT with fused row-sum (Z) accumulation
    and per-slab row-max partials on DVE.
  phase 2a (per i-block): 1/Z, row-max of E -> b, h-partial matmul into one
    PSUM bank; then the 2 KB h AllReduce launches (hidden under phase 2b).
  phase 2b (per i-block): PE-transpose E -> E.T, U_A = E.T.T @ q_bf scaled
    by 1/Z, write G blocks (including c*h once the AllReduce lands).

All matmul operands are fp16 (1 cycle/row on PE, like bf16, but 4x finer
rounding); accumulation is fp32 in PSUM; stats are fp32.
"""

import numpy as np

import concourse.bass as bass
import concourse.mybir as mybir
import concourse.tile as tile
from concourse import bacc
from concourse.bass_utils import run_bass_kernel_spmd
from concourse.masks import make_identity

F32 = mybir.dt.float32
# fp16 (10-bit mantissa) runs matmuls at the same 1 cycle/row as bf16 but
# with 4x finer rounding; E = exp(S) <= e^6 stays far below fp16 max.
BF16 = mybir.dt.float16
AF = mybir.ActivationFunctionType

T = 4096
D = 512
NCORES = 8
TL = T // NCORES          # 512 local context rows per core
P = 128
NIB = TL // P             # 4 i-blocks of 128 rows
NJT = T // P              # 32 j-tiles of 128
NJS = T // 512            # 8 j-slabs of 512
NDC = D // P              # 4 d-chunks of 128


def build_kernel(collective=True, compile=True):
    nc = bacc.Bacc("TRN2", target_bir_lowering=False, debug=False,
                   num_devices=NCORES if collective else 1)

    c_dram = nc.dram_tensor("c", [TL, D], F32, kind="ExternalInput").ap()
    q_dram = nc.dram_tensor("q", [T, D], F32, kind="ExternalInput").ap()
    w2p_dram = nc.dram_tensor("w2p", [P, NDC], F32, kind="ExternalInput").ap()
    w3p_dram = nc.dram_tensor("w3p", [P, NDC], F32, kind="ExternalInput").ap()
    g_dram = nc.dram_tensor("g", [TL, 4 * D], F32, kind="ExternalOutput").ap()

    with tile.TileContext(nc) as tc:
        _emit(nc, tc, c_dram, q_dram, w2p_dram, w3p_dram, g_dram,
              collective=collective)

    if compile:
        nc.compile()
    return nc


def _emit(nc, tc, c_dram, q_dram, w2p_dram, w3p_dram, g_dram,
          collective=True):
    from contextlib import ExitStack
    ctx = ExitStack()
    consts = ctx.enter_context(tc.tile_pool(name="consts", bufs=1))
    epool = ctx.enter_context(tc.tile_pool(name="epool", bufs=1))
    etpool = ctx.enter_context(tc.tile_pool(name="etpool", bufs=2))
    spool = ctx.enter_context(tc.tile_pool(name="spool", bufs=2, space="PSUM"))
    tppool = ctx.enter_context(tc.tile_pool(name="tppool", bufs=4, space="PSUM"))
    uapool = ctx.enter_context(tc.tile_pool(name="uapool", bufs=1, space="PSUM"))
    miscps = ctx.enter_context(tc.tile_pool(name="miscps", bufs=1, space="PSUM"))
    stat = ctx.enter_context(tc.tile_pool(name="stat", bufs=4))
    gout = ctx.enter_context(tc.tile_pool(name="gout", bufs=3))
    dram = ctx.enter_context(tc.tile_pool(name="dram", bufs=1, space="DRAM"))

    # ---- prologue ---------------------------------------------------------
    # ident first: it is tiny gpsimd work but gates every PE transpose, and
    # the gpsimd (Pool) queue also generates all SWDGE cast-DMA descriptors.
    q_bf = consts.tile([P, NJS, 4, D], BF16)  # [p, js, k, d] ; jt = 4*js+k
    ident = consts.tile([P, P], BF16)
    make_identity(nc, ident)
    ones1 = consts.tile([1, P], BF16)
    nc.gpsimd.memset(ones1, 1.0)

    w2p = consts.tile([P, NDC], F32)
    nc.sync.dma_start(out=w2p, in_=w2p_dram)
    w2pb = consts.tile([P, NDC], BF16)
    nc.vector.tensor_copy(out=w2pb, in_=w2p)
    w3p = consts.tile([P, NDC], F32)
    nc.sync.dma_start(out=w3p, in_=w3p_dram)

    qT = []  # qT[dc]: (128 d, 4096 j) bf16
    for dc in range(NDC):
        qT.append(consts.tile([P, T], BF16, tag=f"qT{dc}", name=f"qT{dc}"))
    q2r = consts.tile([1, T], BF16)

    def emit_slab_transposes(js):
        for dc in range(NDC):
            ps = tppool.tile([P, 512], BF16, tag="tp", name=f"tq{js}{dc}")
            for k in range(4):
                nc.tensor.transpose(ps[:, k * P:(k + 1) * P],
                                    q_bf[:, js, k, dc * P:(dc + 1) * P],
                                    ident)
            nc.vector.tensor_copy(out=qT[dc][:, js * 512:(js + 1) * 512],
                                  in_=ps)

    def emit_q2r(js):
        psq = miscps.tile([1, 512], F32, tag="small", name=f"q2ps{js}")
        for dc in range(NDC):
            nc.tensor.matmul(psq, lhsT=w2pb[:, dc:dc + 1],
                             rhs=qT[dc][:, js * 512:(js + 1) * 512],
                             start=(dc == 0), stop=(dc == NDC - 1))
        nc.scalar.activation(out=q2r[:, js * 512:(js + 1) * 512], in_=psq,
                             func=AF.Copy)

    # ---- context: load f32, one SWDGE cast-load bf16 ---------------------
    c_nat = []
    for ib in range(NIB):
        t = consts.tile([P, D], F32, tag=f"c_nat{ib}", name=f"c_nat{ib}")
        nc.sync.dma_start(out=t, in_=c_dram[ib * P:(ib + 1) * P, :])
        c_nat.append(t)
    c_bf = consts.tile([P, NIB, D], BF16)  # [p, ib, d]
    nc.gpsimd.dma_start(out=c_bf,
                        in_=c_dram.rearrange("(ib p) d -> p ib d", p=P))

    cw3T = []  # cw3T[dc]: (128 d, 512 i) bf16 = (context*w3).T chunk
    for dc in range(NDC):
        ps = tppool.tile([P, TL], BF16, tag="tp")
        for ib in range(NIB):
            nc.tensor.transpose(ps[:, ib * P:(ib + 1) * P],
                                c_bf[:, ib, dc * P:(dc + 1) * P], ident)
        t = consts.tile([P, TL], BF16, tag=f"cw3T{dc}", name=f"cw3T{dc}")
        nc.scalar.activation(out=t, in_=ps, func=AF.Copy,
                             scale=w3p[:, dc:dc + 1])
        cw3T.append(t)

    # ---- persistent per-i-block E, Z-partial and max-partial buffers -----
    e_sb = []
    zpart = []
    mpart = []
    for ib in range(NIB):
        e_sb.append(epool.tile([P, T], BF16, tag=f"e{ib}", name=f"e{ib}"))
        zpart.append(stat.tile([P, NJS], F32, tag=f"zp{ib}", name=f"zp{ib}"))
        mpart.append(stat.tile([P, NJS], F32, tag=f"mp{ib}", name=f"mp{ib}"))

    # ---- phase 1: per j-slab pipeline ------------------------------------
    for js in range(NJS):
        # cast-load one 512-row slab of question as bf16
        nc.gpsimd.dma_start(
            out=q_bf[:, js],
            in_=q_dram[js * 512:(js + 1) * 512, :]
                .rearrange("(k p) d -> p k d", p=P))
        emit_slab_transposes(js)
        emit_q2r(js)
        # S and E for all four i-blocks on this slab
        for ib in range(NIB):
            ps = spool.tile([P, 512], F32, tag="s")
            for dc in range(NDC):
                nc.tensor.matmul(ps, lhsT=cw3T[dc][:, ib * P:(ib + 1) * P],
                                 rhs=qT[dc][:, js * 512:(js + 1) * 512],
                                 start=(dc == 0), stop=False)
            nc.tensor.matmul(ps, lhsT=ones1,
                             rhs=q2r[:, js * 512:(js + 1) * 512],
                             start=False, stop=True)
            nc.scalar.activation(out=e_sb[ib][:, js * 512:(js + 1) * 512],
                                 in_=ps, func=AF.Exp,
                                 accum_out=zpart[ib][:, js:js + 1])
            nc.vector.tensor_reduce(out=mpart[ib][:, js:js + 1],
                                    in_=e_sb[ib][:, js * 512:(js + 1) * 512],
                                    axis=mybir.AxisListType.X,
                                    op=mybir.AluOpType.max)

    # ---- phase 2a: per i-block stats + h partial, launch AllReduce -------
    h_ps = miscps.tile([P, NDC], F32, tag="small")  # h partial accumulator
    zinvs = []
    for ib in range(NIB):
        z = stat.tile([P, 1], F32, tag="z")
        nc.vector.tensor_reduce(out=z, in_=zpart[ib],
                                axis=mybir.AxisListType.X,
                                op=mybir.AluOpType.add)
        zinv = stat.tile([P, 1], F32, tag=f"zinv{ib}", name=f"zinv{ib}")
        nc.vector.reciprocal(out=zinv, in_=z)
        zinvs.append(zinv)
        maxe = stat.tile([P, 1], F32, tag="maxe")
        nc.vector.tensor_reduce(out=maxe, in_=mpart[ib],
                                axis=mybir.AxisListType.X,
                                op=mybir.AluOpType.max)
        b = stat.tile([P, 1], F32, tag="b")
        nc.vector.tensor_mul(out=b, in0=maxe, in1=zinv)
        b_bf = stat.tile([P, 1], BF16, tag="b_bf")
        nc.vector.tensor_copy(out=b_bf, in_=b)

        # h partial: h[dc] += c_bf[:, ib, dc].T @ b
        # NOTE start=True clears has_written for the WHOLE bank, so only the
        # very first matmul touching this bank may set it.
        for dc in range(NDC):
            nc.tensor.matmul(h_ps[:, dc:dc + 1],
                             lhsT=c_bf[:, ib, dc * P:(dc + 1) * P],
                             rhs=b_bf,
                             start=(ib == 0 and dc == 0),
                             stop=(ib == NIB - 1 and dc == NDC - 1),
                             skip_group_check=True)

    # h AllReduce launches here; it overlaps the U_A phase below.
    h_sb = stat.tile([P, NDC], F32, tag="h_sb")
    nc.scalar.activation(out=h_sb, in_=h_ps, func=AF.Copy)
    hp_dram = dram.tile([D], F32)
    hs_dram = dram.tile([D], F32)
    hp_ap = hp_dram[:]
    nc.sync.dma_start(out=hp_ap.rearrange("(dc p) -> p dc", p=P), in_=h_sb)
    if collective:
        nc.gpsimd.collective_compute(
            "AllReduce", mybir.AluOpType.add,
            replica_groups=[list(range(NCORES))],
            ins=[hp_dram.opt()], outs=[hs_dram.opt()],
        )
    else:
        nc.sync.dma_start(out=hs_dram[:], in_=hp_dram[:])
    hs_ap = hs_dram[:]
    h_bc = consts.tile([P, D], F32)
    nc.sync.dma_start(
        out=h_bc,
        in_=bass.AP(tensor=hs_ap.tensor, offset=hs_ap.offset,
                    ap=[[0, P], [1, D]]),
    )

    # ---- phase 2b: per i-block E.T, U_A, G -------------------------------
    for ib in range(NIB):
        # G block 0 does not depend on anything but the c load
        nc.sync.dma_start(out=g_dram[ib * P:(ib + 1) * P, 0:D], in_=c_nat[ib])

        # E.T via PE transposes; copies on DVE; U_A matmuls follow per group
        et_sb = etpool.tile([P, T], BF16, tag="et")
        ua_ps = uapool.tile([P, D], F32, tag="ua")
        for jg in range(NJS):
            ps = tppool.tile([P, 512], BF16, tag="tp")
            for k in range(4):
                jt = jg * 4 + k
                nc.tensor.transpose(ps[:, k * P:(k + 1) * P],
                                    e_sb[ib][:, jt * P:(jt + 1) * P], ident)
            nc.vector.tensor_copy(out=et_sb[:, jg * 512:(jg + 1) * 512],
                                  in_=ps)
            for k in range(4):
                jc = jg * 4 + k
                nc.tensor.matmul(ua_ps,
                                 lhsT=et_sb[:, jc * P:(jc + 1) * P],
                                 rhs=q_bf[:, jg, k, :],
                                 start=(jc == 0), stop=(jc == NJT - 1))
        ua = gout.tile([P, D], F32, tag="ua_sb")
        nc.scalar.activation(out=ua, in_=ua_ps, func=AF.Copy, scale=zinvs[ib])

        # G blocks 1..2
        nc.sync.dma_start(out=g_dram[ib * P:(ib + 1) * P, D:2 * D], in_=ua)
        cu = gout.tile([P, D], F32, tag="cu")
        nc.vector.tensor_mul(out=cu, in0=c_nat[ib], in1=ua)
        nc.sync.dma_start(out=g_dram[ib * P:(ib + 1) * P, 2 * D:3 * D], in_=cu)

        # G block 3 (c*h) — h_bc arrives while U_A runs
        ch = gout.tile([P, D], F32, tag="ch")
        nc.vector.tensor_mul(out=ch, in0=c_nat[ib], in1=h_bc)
        nc.sync.dma_start(out=g_dram[ib * P:(ib + 1) * P, 3 * D:4 * D], in_=ch)

    ctx.close()


_NC_CACHE = {}


def _get_nc():
    if "nc" not in _NC_CACHE:
        _NC_CACHE["nc"] = build_kernel()
    return _NC_CACHE["nc"]


def kernel(x: np.ndarray, kernel: np.ndarray) -> np.ndarray:
    nc = _get_nc()

    context = np.ascontiguousarray(x[0, 0]).astype(np.float32)   # (T, D)
    question = np.ascontiguousarray(x[1, 0]).astype(np.float32)  # (T, D)
    w = np.asarray(kernel, dtype=np.float32)
    w2 = w[D:2 * D]
    w3 = w[2 * D:3 * D]
    # partition-major chunk layout: wp[p, dc] = w[dc*128 + p]
    w2p = np.ascontiguousarray(w2.reshape(NDC, P).T)
    w3p = np.ascontiguousarray(w3.reshape(NDC, P).T)

    in_maps = []
    for core in range(NCORES):
        in_maps.append({
            "c": np.ascontiguousarray(context[core * TL:(core + 1) * TL]),
            "q": question,
            "w2p": w2p,
            "w3p": w3p,
        })

    res = run_bass_kernel_spmd(nc, in_maps, core_ids=list(range(NCORES)))
    g = np.concatenate([res.results[core]["g"] for core in range(NCORES)],
                       axis=0)
    return g.astype(np.float32)


# revision 44
# speedup vs baseline: 1.0025x; 1.0025x over previous
"""BiAttention (BiDAF-style) kernel for Trainium2, 8 NeuronCores.

Reference math (T=4096, d=512):
    context  = x[0,0]; question = x[1,0]
    S[i,j]   = w1.c_i + w2.q_j + (c_i*w3).q_j
    A        = softmax_j(S)          # w1.c_i is constant per row -> cancels
    U_A      = A @ question
    b        = max_j A[i,j]          # == max_j E[i,j] / Z_i  with E=exp(S)
    h        = b @ context           # global over T -> one AllReduce
    G        = [context, U_A, context*U_A, context*h]

Sharding: context rows (and rows of S/A/U_A/G) split across 8 cores
(512 rows each); question replicated; h all-reduced (2 KB).

Per-core schedule:
  phase 1 (per 512-wide j-slab): SWDGE cast-load q slab (fp16), PE-transpose
    to qT, q2 row via matmul, S = cw3T.T @ qT + q2 (K=1 ones-fold matmul)
    for all four i-blocks, exp on ACT with fused row-sum (Z) accumulation
    and per-slab row-max partials on DVE.
  phase 2a (per i-block): 1/Z, row-max of E -> b, h-partial matmul into one
    PSUM bank; then the 2 KB h AllReduce launches (hidden under phase 2b).
  phase 2b (per i-block): PE-transpose E -> E.T, U_A = E.T.T @ q_bf scaled
    by 1/Z, write G blocks (including c*h once the AllReduce lands).

All matmul operands are fp16 (1 cycle/row on PE, like bf16, but 4x finer
rounding); accumulation is fp32 in PSUM; stats are fp32.
"""

import numpy as np

import concourse.bass as bass
import concourse.mybir as mybir
import concourse.tile as tile
from concourse import bacc
from concourse.bass_utils import run_bass_kernel_spmd
from concourse.masks import make_identity

F32 = mybir.dt.float32
# fp16 (10-bit mantissa) runs matmuls at the same 1 cycle/row as bf16 but
# with 4x finer rounding; E = exp(S) <= e^6 stays far below fp16 max.
BF16 = mybir.dt.float16
AF = mybir.ActivationFunctionType

T = 4096
D = 512
NCORES = 8
TL = T // NCORES          # 512 local context rows per core
P = 128
NIB = TL // P             # 4 i-blocks of 128 rows
NJT = T // P              # 32 j-tiles of 128
NJS = T // 512            # 8 j-slabs of 512
NDC = D // P              # 4 d-chunks of 128


def build_kernel(collective=True, compile=True):
    nc = bacc.Bacc("TRN2", target_bir_lowering=False, debug=False,
                   num_devices=NCORES if collective else 1)

    c_dram = nc.dram_tensor("c", [TL, D], F32, kind="ExternalInput").ap()
    q_dram = nc.dram_tensor("q", [T, D], F32, kind="ExternalInput").ap()
    w2p_dram = nc.dram_tensor("w2p", [P, NDC], F32, kind="ExternalInput").ap()
    w3p_dram = nc.dram_tensor("w3p", [P, NDC], F32, kind="ExternalInput").ap()
    g_dram = nc.dram_tensor("g", [TL, 4 * D], F32, kind="ExternalOutput").ap()

    with tile.TileContext(nc) as tc:
        _emit(nc, tc, c_dram, q_dram, w2p_dram, w3p_dram, g_dram,
              collective=collective)

    if compile:
        nc.compile()
    return nc


def _emit(nc, tc, c_dram, q_dram, w2p_dram, w3p_dram, g_dram,
          collective=True):
    from contextlib import ExitStack
    ctx = ExitStack()
    consts = ctx.enter_context(tc.tile_pool(name="consts", bufs=1))
    epool = ctx.enter_context(tc.tile_pool(name="epool", bufs=1))
    etpool = ctx.enter_context(tc.tile_pool(name="etpool", bufs=2))
    spool = ctx.enter_context(tc.tile_pool(name="spool", bufs=2, space="PSUM"))
    tppool = ctx.enter_context(tc.tile_pool(name="tppool", bufs=4, space="PSUM"))
    uapool = ctx.enter_context(tc.tile_pool(name="uapool", bufs=1, space="PSUM"))
    miscps = ctx.enter_context(tc.tile_pool(name="miscps", bufs=1, space="PSUM"))
    stat = ctx.enter_context(tc.tile_pool(name="stat", bufs=4))
    gout = ctx.enter_context(tc.tile_pool(name="gout", bufs=3))
    dram = ctx.enter_context(tc.tile_pool(name="dram", bufs=1, space="DRAM"))

    # ---- prologue ---------------------------------------------------------
    # ident first: it is tiny gpsimd work but gates every PE transpose, and
    # the gpsimd (Pool) queue also generates all SWDGE cast-DMA descriptors.
    q_bf = consts.tile([P, NJS, 4, D], BF16)  # [p, js, k, d] ; jt = 4*js+k
    ident = consts.tile([P, P], BF16)
    make_identity(nc, ident)
    ones1 = consts.tile([1, P], BF16)
    nc.gpsimd.memset(ones1, 1.0)

    # dummy exp: pull the ~2.7us ACT table load for exp_and_others into the
    # startup DMA-wait window instead of stalling the first real exp
    warm = consts.tile([1, 1], F32)
    nc.vector.memset(warm, 0.0)
    nc.scalar.activation(out=warm, in_=warm, func=AF.Exp)

    w2p = consts.tile([P, NDC], F32)
    nc.sync.dma_start(out=w2p, in_=w2p_dram)
    w2pb = consts.tile([P, NDC], BF16)
    nc.vector.tensor_copy(out=w2pb, in_=w2p)
    w3p = consts.tile([P, NDC], F32)
    nc.sync.dma_start(out=w3p, in_=w3p_dram)

    qT = []  # qT[dc]: (128 d, 4096 j) bf16
    for dc in range(NDC):
        qT.append(consts.tile([P, T], BF16, tag=f"qT{dc}", name=f"qT{dc}"))
    q2r = consts.tile([1, T], BF16)

    def emit_slab_transposes(js):
        for dc in range(NDC):
            ps = tppool.tile([P, 512], BF16, tag="tp", name=f"tq{js}{dc}")
            for k in range(4):
                nc.tensor.transpose(ps[:, k * P:(k + 1) * P],
                                    q_bf[:, js, k, dc * P:(dc + 1) * P],
                                    ident)
            nc.vector.tensor_copy(out=qT[dc][:, js * 512:(js + 1) * 512],
                                  in_=ps)

    def emit_q2r(js):
        psq = miscps.tile([1, 512], F32, tag="small", name=f"q2ps{js}")
        for dc in range(NDC):
            nc.tensor.matmul(psq, lhsT=w2pb[:, dc:dc + 1],
                             rhs=qT[dc][:, js * 512:(js + 1) * 512],
                             start=(dc == 0), stop=(dc == NDC - 1))
        nc.scalar.activation(out=q2r[:, js * 512:(js + 1) * 512], in_=psq,
                             func=AF.Copy)

    # ---- context: load f32, one SWDGE cast-load bf16 ---------------------
    c_nat = []
    for ib in range(NIB):
        t = consts.tile([P, D], F32, tag=f"c_nat{ib}", name=f"c_nat{ib}")
        nc.sync.dma_start(out=t, in_=c_dram[ib * P:(ib + 1) * P, :])
        c_nat.append(t)
    c_bf = consts.tile([P, NIB, D], BF16)  # [p, ib, d]
    nc.gpsimd.dma_start(out=c_bf,
                        in_=c_dram.rearrange("(ib p) d -> p ib d", p=P))

    cw3T = []  # cw3T[dc]: (128 d, 512 i) bf16 = (context*w3).T chunk
    for dc in range(NDC):
        ps = tppool.tile([P, TL], BF16, tag="tp")
        for ib in range(NIB):
            nc.tensor.transpose(ps[:, ib * P:(ib + 1) * P],
                                c_bf[:, ib, dc * P:(dc + 1) * P], ident)
        t = consts.tile([P, TL], BF16, tag=f"cw3T{dc}", name=f"cw3T{dc}")
        nc.scalar.activation(out=t, in_=ps, func=AF.Copy,
                             scale=w3p[:, dc:dc + 1])
        cw3T.append(t)

    # ---- persistent per-i-block E, Z-partial and max-partial buffers -----
    e_sb = []
    zpart = []
    mpart = []
    for ib in range(NIB):
        e_sb.append(epool.tile([P, T], BF16, tag=f"e{ib}", name=f"e{ib}"))
        zpart.append(stat.tile([P, NJS], F32, tag=f"zp{ib}", name=f"zp{ib}"))
        mpart.append(stat.tile([P, NJS], F32, tag=f"mp{ib}", name=f"mp{ib}"))

    # ---- phase 1: per j-slab pipeline ------------------------------------
    for js in range(NJS):
        # cast-load one 512-row slab of question as bf16
        nc.gpsimd.dma_start(
            out=q_bf[:, js],
            in_=q_dram[js * 512:(js + 1) * 512, :]
                .rearrange("(k p) d -> p k d", p=P))
        emit_slab_transposes(js)
        emit_q2r(js)
        # S and E for all four i-blocks on this slab
        for ib in range(NIB):
            ps = spool.tile([P, 512], F32, tag="s")
            for dc in range(NDC):
                nc.tensor.matmul(ps, lhsT=cw3T[dc][:, ib * P:(ib + 1) * P],
                                 rhs=qT[dc][:, js * 512:(js + 1) * 512],
                                 start=(dc == 0), stop=False)
            nc.tensor.matmul(ps, lhsT=ones1,
                             rhs=q2r[:, js * 512:(js + 1) * 512],
                             start=False, stop=True)
            nc.scalar.activation(out=e_sb[ib][:, js * 512:(js + 1) * 512],
                                 in_=ps, func=AF.Exp,
                                 accum_out=zpart[ib][:, js:js + 1])
            nc.vector.tensor_reduce(out=mpart[ib][:, js:js + 1],
                                    in_=e_sb[ib][:, js * 512:(js + 1) * 512],
                                    axis=mybir.AxisListType.X,
                                    op=mybir.AluOpType.max)

    # ---- phase 2a: per i-block stats + h partial, launch AllReduce -------
    h_ps = miscps.tile([P, NDC], F32, tag="small")  # h partial accumulator
    zinvs = []
    for ib in range(NIB):
        z = stat.tile([P, 1], F32, tag="z")
        nc.vector.tensor_reduce(out=z, in_=zpart[ib],
                                axis=mybir.AxisListType.X,
                                op=mybir.AluOpType.add)
        zinv = stat.tile([P, 1], F32, tag=f"zinv{ib}", name=f"zinv{ib}")
        nc.vector.reciprocal(out=zinv, in_=z)
        zinvs.append(zinv)
        maxe = stat.tile([P, 1], F32, tag="maxe")
        nc.vector.tensor_reduce(out=maxe, in_=mpart[ib],
                                axis=mybir.AxisListType.X,
                                op=mybir.AluOpType.max)
        b = stat.tile([P, 1], F32, tag="b")
        nc.vector.tensor_mul(out=b, in0=maxe, in1=zinv)
        b_bf = stat.tile([P, 1], BF16, tag="b_bf")
        nc.vector.tensor_copy(out=b_bf, in_=b)

        # h partial: h[dc] += c_bf[:, ib, dc].T @ b
        # NOTE start=True clears has_written for the WHOLE bank, so only the
        # very first matmul touching this bank may set it.
        for dc in range(NDC):
            nc.tensor.matmul(h_ps[:, dc:dc + 1],
                             lhsT=c_bf[:, ib, dc * P:(dc + 1) * P],
                             rhs=b_bf,
                             start=(ib == 0 and dc == 0),
                             stop=(ib == NIB - 1 and dc == NDC - 1),
                             skip_group_check=True)

    # h AllReduce launches here; it overlaps the U_A phase below.
    h_sb = stat.tile([P, NDC], F32, tag="h_sb")
    nc.scalar.activation(out=h_sb, in_=h_ps, func=AF.Copy)
    hp_dram = dram.tile([D], F32)
    hs_dram = dram.tile([D], F32)
    hp_ap = hp_dram[:]
    nc.sync.dma_start(out=hp_ap.rearrange("(dc p) -> p dc", p=P), in_=h_sb)
    if collective:
        nc.gpsimd.collective_compute(
            "AllReduce", mybir.AluOpType.add,
            replica_groups=[list(range(NCORES))],
            ins=[hp_dram.opt()], outs=[hs_dram.opt()],
        )
    else:
        nc.sync.dma_start(out=hs_dram[:], in_=hp_dram[:])
    hs_ap = hs_dram[:]
    h_bc = consts.tile([P, D], F32)
    nc.sync.dma_start(
        out=h_bc,
        in_=bass.AP(tensor=hs_ap.tensor, offset=hs_ap.offset,
                    ap=[[0, P], [1, D]]),
    )

    # ---- phase 2b: per i-block E.T, U_A, G -------------------------------
    for ib in range(NIB):
        # G block 0 does not depend on anything but the c load
        nc.sync.dma_start(out=g_dram[ib * P:(ib + 1) * P, 0:D], in_=c_nat[ib])

        # E.T via PE transposes; copies on DVE; U_A matmuls follow per group
        et_sb = etpool.tile([P, T], BF16, tag="et")
        ua_ps = uapool.tile([P, D], F32, tag="ua")
        for jg in range(NJS):
            ps = tppool.tile([P, 512], BF16, tag="tp")
            for k in range(4):
                jt = jg * 4 + k
                nc.tensor.transpose(ps[:, k * P:(k + 1) * P],
                                    e_sb[ib][:, jt * P:(jt + 1) * P], ident)
            nc.vector.tensor_copy(out=et_sb[:, jg * 512:(jg + 1) * 512],
                                  in_=ps)
            for k in range(4):
                jc = jg * 4 + k
                nc.tensor.matmul(ua_ps,
                                 lhsT=et_sb[:, jc * P:(jc + 1) * P],
                                 rhs=q_bf[:, jg, k, :],
                                 start=(jc == 0), stop=(jc == NJT - 1))
        ua = gout.tile([P, D], F32, tag="ua_sb")
        nc.scalar.activation(out=ua, in_=ua_ps, func=AF.Copy, scale=zinvs[ib])

        # G blocks 1..2
        nc.sync.dma_start(out=g_dram[ib * P:(ib + 1) * P, D:2 * D], in_=ua)
        cu = gout.tile([P, D], F32, tag="cu")
        nc.vector.tensor_mul(out=cu, in0=c_nat[ib], in1=ua)
        nc.sync.dma_start(out=g_dram[ib * P:(ib + 1) * P, 2 * D:3 * D], in_=cu)

        # G block 3 (c*h) — h_bc arrives while U_A runs
        ch = gout.tile([P, D], F32, tag="ch")
        nc.vector.tensor_mul(out=ch, in0=c_nat[ib], in1=h_bc)
        nc.sync.dma_start(out=g_dram[ib * P:(ib + 1) * P, 3 * D:4 * D], in_=ch)

    ctx.close()


_NC_CACHE = {}


def _get_nc():
    if "nc" not in _NC_CACHE:
        _NC_CACHE["nc"] = build_kernel()
    return _NC_CACHE["nc"]


def kernel(x: np.ndarray, kernel: np.ndarray) -> np.ndarray:
    nc = _get_nc()

    context = np.ascontiguousarray(x[0, 0]).astype(np.float32)   # (T, D)
    question = np.ascontiguousarray(x[1, 0]).astype(np.float32)  # (T, D)
    w = np.asarray(kernel, dtype=np.float32)
    w2 = w[D:2 * D]
    w3 = w[2 * D:3 * D]
    # partition-major chunk layout: wp[p, dc] = w[dc*128 + p]
    w2p = np.ascontiguousarray(w2.reshape(NDC, P).T)
    w3p = np.ascontiguousarray(w3.reshape(NDC, P).T)

    in_maps = []
    for core in range(NCORES):
        in_maps.append({
            "c": np.ascontiguousarray(context[core * TL:(core + 1) * TL]),
            "q": question,
            "w2p": w2p,
            "w3p": w3p,
        })

    res = run_bass_kernel_spmd(nc, in_maps, core_ids=list(range(NCORES)))
    g = np.concatenate([res.results[core]["g"] for core in range(NCORES)],
                       axis=0)
    return g.astype(np.float32)


# revision 50
# speedup vs baseline: 1.1656x; 1.1627x over previous
"""BiAttention (BiDAF-style) kernel for Trainium2, 8 NeuronCores.

Reference math (T=4096, d=512):
    context  = x[0,0]; question = x[1,0]
    S[i,j]   = w1.c_i + w2.q_j + (c_i*w3).q_j
    A        = softmax_j(S)          # w1.c_i is constant per row -> cancels
    U_A      = A @ question
    b        = max_j A[i,j]          # == max_j E[i,j] / Z_i  with E=exp(S)
    h        = b @ context           # global over T -> one AllReduce
    G        = [context, U_A, context*U_A, context*h]

Sharding: context rows (and rows of S/A/U_A/G) split across 8 cores
(512 rows each); question replicated; h all-reduced (2 KB).

Per-core schedule:
  phase 1 (per 512-wide j-slab): SWDGE cast-load q slab (fp16), PE-transpose
    to qT, then S = W.T @ qT for all four i-blocks where the stationary
    W[dc] = (c*w3).T[dc] + w2[dc] carries the q2 bias for free (because
    sum_dc sum_k w2[k]*qT[dc][k,j] = q2[j]); exp on ACT with fused row-sum
    (Z) accumulation and per-slab row-max partials on DVE.
  phase 2a (per i-block): 1/Z, row-max of E -> b, h-partial matmul into one
    PSUM bank; then the 2 KB h AllReduce launches (hidden under phase 2b).
  phase 2b (per i-block): PE-transpose E -> E.T, U_A = E.T.T @ q_bf scaled
    by 1/Z, write G blocks (including c*h once the AllReduce lands).

All matmul operands are fp16 (1 cycle/row on PE, like bf16, but 4x finer
rounding); accumulation is fp32 in PSUM; stats are fp32.
"""

import numpy as np

import concourse.bass as bass
import concourse.mybir as mybir
import concourse.tile as tile
from concourse import bacc
from concourse.bass_utils import run_bass_kernel_spmd
from concourse.masks import make_identity

F32 = mybir.dt.float32
# fp16 (10-bit mantissa) runs matmuls at the same 1 cycle/row as bf16 but
# with 4x finer rounding; E = exp(S) <= e^6 stays far below fp16 max.
BF16 = mybir.dt.float16
AF = mybir.ActivationFunctionType

T = 4096
D = 512
NCORES = 8
TL = T // NCORES          # 512 local context rows per core
P = 128
NIB = TL // P             # 4 i-blocks of 128 rows
NJT = T // P              # 32 j-tiles of 128
NJS = T // 512            # 8 j-slabs of 512
NDC = D // P              # 4 d-chunks of 128


def build_kernel(collective=True, compile=True):
    nc = bacc.Bacc("TRN2", target_bir_lowering=False, debug=False,
                   num_devices=NCORES if collective else 1)

    c_dram = nc.dram_tensor("c", [TL, D], F32, kind="ExternalInput").ap()
    q_dram = nc.dram_tensor("q", [T, D], F32, kind="ExternalInput").ap()
    w2p_dram = nc.dram_tensor("w2p", [P, NDC], F32, kind="ExternalInput").ap()
    w3p_dram = nc.dram_tensor("w3p", [P, NDC], F32, kind="ExternalInput").ap()
    g_dram = nc.dram_tensor("g", [TL, 4 * D], F32, kind="ExternalOutput").ap()

    with tile.TileContext(nc) as tc:
        _emit(nc, tc, c_dram, q_dram, w2p_dram, w3p_dram, g_dram,
              collective=collective)

    if compile:
        nc.compile()
    return nc


def _emit(nc, tc, c_dram, q_dram, w2p_dram, w3p_dram, g_dram,
          collective=True):
    from contextlib import ExitStack
    ctx = ExitStack()
    consts = ctx.enter_context(tc.tile_pool(name="consts", bufs=1))
    epool = ctx.enter_context(tc.tile_pool(name="epool", bufs=1))
    etpool = ctx.enter_context(tc.tile_pool(name="etpool", bufs=2))
    spool = ctx.enter_context(tc.tile_pool(name="spool", bufs=2, space="PSUM"))
    tppool = ctx.enter_context(tc.tile_pool(name="tppool", bufs=4, space="PSUM"))
    uapool = ctx.enter_context(tc.tile_pool(name="uapool", bufs=1, space="PSUM"))
    miscps = ctx.enter_context(tc.tile_pool(name="miscps", bufs=1, space="PSUM"))
    stat = ctx.enter_context(tc.tile_pool(name="stat", bufs=4))
    gout = ctx.enter_context(tc.tile_pool(name="gout", bufs=3))
    dram = ctx.enter_context(tc.tile_pool(name="dram", bufs=1, space="DRAM"))

    # ---- prologue ---------------------------------------------------------
    # ident first: it is tiny gpsimd work but gates every PE transpose, and
    # the gpsimd (Pool) queue also generates all SWDGE cast-DMA descriptors.
    q_bf = consts.tile([P, NJS, 4, D], BF16)  # [p, js, k, d] ; jt = 4*js+k
    # c_bf cast-DMA descriptor first: its transfer overlaps ident setup and
    # it gates PE's first work (the cw3T transposes)
    c_bf = consts.tile([P, NIB, D], BF16)  # [p, ib, d]
    nc.gpsimd.dma_start(out=c_bf,
                        in_=c_dram.rearrange("(ib p) d -> p ib d", p=P))
    ident = consts.tile([P, P], BF16)
    make_identity(nc, ident)
    # dummy exp: pull the ~2.7us ACT table load for exp_and_others into the
    # startup DMA-wait window instead of stalling the first real exp
    warm = consts.tile([1, 1], F32)
    nc.vector.memset(warm, 0.0)
    nc.scalar.activation(out=warm, in_=warm, func=AF.Exp)

    w2p = consts.tile([P, NDC], F32)
    nc.sync.dma_start(out=w2p, in_=w2p_dram)
    w3p = consts.tile([P, NDC], F32)
    nc.sync.dma_start(out=w3p, in_=w3p_dram)

    qT = []  # qT[dc]: (128 d, 4096 j) bf16
    for dc in range(NDC):
        qT.append(consts.tile([P, T], BF16, tag=f"qT{dc}", name=f"qT{dc}"))

    def emit_slab_transposes(js):
        for dc in range(NDC):
            ps = tppool.tile([P, 512], BF16, tag="tp", name=f"tq{js}{dc}")
            for k in range(4):
                nc.tensor.transpose(ps[:, k * P:(k + 1) * P],
                                    q_bf[:, js, k, dc * P:(dc + 1) * P],
                                    ident)
            nc.vector.tensor_copy(out=qT[dc][:, js * 512:(js + 1) * 512],
                                  in_=ps)

    # ---- context: load f32 ------------------------------------------------
    c_nat = []
    for ib in range(NIB):
        t = consts.tile([P, D], F32, tag=f"c_nat{ib}", name=f"c_nat{ib}")
        nc.sync.dma_start(out=t, in_=c_dram[ib * P:(ib + 1) * P, :])
        c_nat.append(t)

    # cw3T[dc] = (context * w3).T chunk PLUS the w2 bias row-constant:
    # W[dc][k,i] = c[i, dc*128+k]*w3[dc*128+k] + w2[dc*128+k].  Because
    #   sum_dc sum_k w2[dc*128+k] * qT[dc][k,j] = (q @ w2)[j] = q2[j],
    # the S matmul then produces  S = (c*w3) @ q.T + q2  directly — the q2
    # bias costs zero extra matmuls (folded into the stationary operand).
    cw3T = []
    for dc in range(NDC):
        ps = tppool.tile([P, TL], BF16, tag="tp")
        for ib in range(NIB):
            nc.tensor.transpose(ps[:, ib * P:(ib + 1) * P],
                                c_bf[:, ib, dc * P:(dc + 1) * P], ident)
        t = consts.tile([P, TL], BF16, tag=f"cw3T{dc}", name=f"cw3T{dc}")
        nc.scalar.activation(out=t, in_=ps, func=AF.Identity,
                             bias=w2p[:, dc:dc + 1],
                             scale=w3p[:, dc:dc + 1])
        cw3T.append(t)

    # ---- persistent per-i-block E, Z-partial and max-partial buffers -----
    e_sb = []
    zpart = []
    mpart = []
    for ib in range(NIB):
        e_sb.append(epool.tile([P, T], BF16, tag=f"e{ib}", name=f"e{ib}"))
        zpart.append(stat.tile([P, NJS], F32, tag=f"zp{ib}", name=f"zp{ib}"))
        mpart.append(stat.tile([P, NJS], F32, tag=f"mp{ib}", name=f"mp{ib}"))

    # ---- phase 1: per j-slab pipeline ------------------------------------
    for js in range(NJS):
        # cast-load one 512-row slab of question as bf16
        nc.gpsimd.dma_start(
            out=q_bf[:, js],
            in_=q_dram[js * 512:(js + 1) * 512, :]
                .rearrange("(k p) d -> p k d", p=P))
        emit_slab_transposes(js)
        # S (with the q2 bias already folded into cw3T) and E per i-block
        for ib in range(NIB):
            ps = spool.tile([P, 512], F32, tag="s")
            for dc in range(NDC):
                nc.tensor.matmul(ps, lhsT=cw3T[dc][:, ib * P:(ib + 1) * P],
                                 rhs=qT[dc][:, js * 512:(js + 1) * 512],
                                 start=(dc == 0), stop=(dc == NDC - 1))
            nc.scalar.activation(out=e_sb[ib][:, js * 512:(js + 1) * 512],
                                 in_=ps, func=AF.Exp,
                                 accum_out=zpart[ib][:, js:js + 1])
            nc.vector.tensor_reduce(out=mpart[ib][:, js:js + 1],
                                    in_=e_sb[ib][:, js * 512:(js + 1) * 512],
                                    axis=mybir.AxisListType.X,
                                    op=mybir.AluOpType.max)

    # ---- phase 2a: per i-block stats + h partial, launch AllReduce -------
    h_ps = miscps.tile([P, NDC], F32, tag="small")  # h partial accumulator
    zinvs = []
    for ib in range(NIB):
        z = stat.tile([P, 1], F32, tag="z")
        nc.vector.tensor_reduce(out=z, in_=zpart[ib],
                                axis=mybir.AxisListType.X,
                                op=mybir.AluOpType.add)
        zinv = stat.tile([P, 1], F32, tag=f"zinv{ib}", name=f"zinv{ib}")
        nc.vector.reciprocal(out=zinv, in_=z)
        zinvs.append(zinv)
        maxe = stat.tile([P, 1], F32, tag="maxe")
        nc.vector.tensor_reduce(out=maxe, in_=mpart[ib],
                                axis=mybir.AxisListType.X,
                                op=mybir.AluOpType.max)
        b = stat.tile([P, 1], F32, tag="b")
        nc.vector.tensor_mul(out=b, in0=maxe, in1=zinv)
        b_bf = stat.tile([P, 1], BF16, tag="b_bf")
        nc.vector.tensor_copy(out=b_bf, in_=b)

        # h partial: h[dc] += c_bf[:, ib, dc].T @ b
        # NOTE start=True clears has_written for the WHOLE bank, so only the
        # very first matmul touching this bank may set it.
        for dc in range(NDC):
            nc.tensor.matmul(h_ps[:, dc:dc + 1],
                             lhsT=c_bf[:, ib, dc * P:(dc + 1) * P],
                             rhs=b_bf,
                             start=(ib == 0 and dc == 0),
                             stop=(ib == NIB - 1 and dc == NDC - 1),
                             skip_group_check=True)

    # h AllReduce launches here; it overlaps the U_A phase below.
    h_sb = stat.tile([P, NDC], F32, tag="h_sb")
    nc.scalar.activation(out=h_sb, in_=h_ps, func=AF.Copy)
    hp_dram = dram.tile([D], F32)
    hs_dram = dram.tile([D], F32)
    hp_ap = hp_dram[:]
    nc.sync.dma_start(out=hp_ap.rearrange("(dc p) -> p dc", p=P), in_=h_sb)
    if collective:
        nc.gpsimd.collective_compute(
            "AllReduce", mybir.AluOpType.add,
            replica_groups=[list(range(NCORES))],
            ins=[hp_dram.opt()], outs=[hs_dram.opt()],
        )
    else:
        nc.sync.dma_start(out=hs_dram[:], in_=hp_dram[:])
    hs_ap = hs_dram[:]
    h_bc = consts.tile([P, D], F32)
    nc.sync.dma_start(
        out=h_bc,
        in_=bass.AP(tensor=hs_ap.tensor, offset=hs_ap.offset,
                    ap=[[0, P], [1, D]]),
    )

    # ---- phase 2b: per i-block E.T, U_A, G -------------------------------
    for ib in range(NIB):
        # G block 0 does not depend on anything but the c load
        nc.sync.dma_start(out=g_dram[ib * P:(ib + 1) * P, 0:D], in_=c_nat[ib])

        # E.T via PE transposes; copies on DVE; U_A matmuls follow per group
        et_sb = etpool.tile([P, T], BF16, tag="et")
        ua_ps = uapool.tile([P, D], F32, tag="ua")
        for jg in range(NJS):
            ps = tppool.tile([P, 512], BF16, tag="tp")
            for k in range(4):
                jt = jg * 4 + k
                nc.tensor.transpose(ps[:, k * P:(k + 1) * P],
                                    e_sb[ib][:, jt * P:(jt + 1) * P], ident)
            nc.vector.tensor_copy(out=et_sb[:, jg * 512:(jg + 1) * 512],
                                  in_=ps)
            for k in range(4):
                jc = jg * 4 + k
                nc.tensor.matmul(ua_ps,
                                 lhsT=et_sb[:, jc * P:(jc + 1) * P],
                                 rhs=q_bf[:, jg, k, :],
                                 start=(jc == 0), stop=(jc == NJT - 1))
        ua = gout.tile([P, D], F32, tag="ua_sb")
        nc.scalar.activation(out=ua, in_=ua_ps, func=AF.Copy, scale=zinvs[ib])

        # G blocks 1..2
        nc.sync.dma_start(out=g_dram[ib * P:(ib + 1) * P, D:2 * D], in_=ua)
        cu = gout.tile([P, D], F32, tag="cu")
        nc.vector.tensor_mul(out=cu, in0=c_nat[ib], in1=ua)
        nc.sync.dma_start(out=g_dram[ib * P:(ib + 1) * P, 2 * D:3 * D], in_=cu)

        # G block 3 (c*h) — h_bc arrives while U_A runs
        ch = gout.tile([P, D], F32, tag="ch")
        nc.vector.tensor_mul(out=ch, in0=c_nat[ib], in1=h_bc)
        nc.sync.dma_start(out=g_dram[ib * P:(ib + 1) * P, 3 * D:4 * D], in_=ch)

    ctx.close()


_NC_CACHE = {}


def _get_nc():
    if "nc" not in _NC_CACHE:
        _NC_CACHE["nc"] = build_kernel()
    return _NC_CACHE["nc"]


def kernel(x: np.ndarray, kernel: np.ndarray) -> np.ndarray:
    nc = _get_nc()

    context = np.ascontiguousarray(x[0, 0]).astype(np.float32)   # (T, D)
    question = np.ascontiguousarray(x[1, 0]).astype(np.float32)  # (T, D)
    w = np.asarray(kernel, dtype=np.float32)
    w2 = w[D:2 * D]
    w3 = w[2 * D:3 * D]
    # partition-major chunk layout: wp[p, dc] = w[dc*128 + p]
    w2p = np.ascontiguousarray(w2.reshape(NDC, P).T)
    w3p = np.ascontiguousarray(w3.reshape(NDC, P).T)

    in_maps = []
    for core in range(NCORES):
        in_maps.append({
            "c": np.ascontiguousarray(context[core * TL:(core + 1) * TL]),
            "q": question,
            "w2p": w2p,
            "w3p": w3p,
        })

    res = run_bass_kernel_spmd(nc, in_maps, core_ids=list(range(NCORES)))
    g = np.concatenate([res.results[core]["g"] for core in range(NCORES)],
                       axis=0)
    return g.astype(np.float32)


# revision 51
# speedup vs baseline: 1.1730x; 1.0063x over previous
"""BiAttention (BiDAF-style) kernel for Trainium2, 8 NeuronCores.

Reference math (T=4096, d=512):
    context  = x[0,0]; question = x[1,0]
    S[i,j]   = w1.c_i + w2.q_j + (c_i*w3).q_j
    A        = softmax_j(S)          # w1.c_i is constant per row -> cancels
    U_A      = A @ question
    b        = max_j A[i,j]          # == max_j E[i,j] / Z_i  with E=exp(S)
    h        = b @ context           # global over T -> one AllReduce
    G        = [context, U_A, context*U_A, context*h]

Sharding: context rows (and rows of S/A/U_A/G) split across 8 cores
(512 rows each); question replicated; h all-reduced (2 KB).

Per-core schedule:
  phase 1 (per 512-wide j-slab): SWDGE cast-load q slab (fp16), PE-transpose
    to qT, then S = W.T @ qT for all four i-blocks where the stationary
    W[dc] = (c*w3).T[dc] + w2[dc] carries the q2 bias for free (because
    sum_dc sum_k w2[k]*qT[dc][k,j] = q2[j]); exp on ACT with fused row-sum
    (Z) accumulation and per-slab row-max partials on DVE.
  phase 2a (per i-block): 1/Z, row-max of E -> b, h-partial matmul into one
    PSUM bank; then the 2 KB h AllReduce launches (hidden under phase 2b).
  phase 2b (per i-block): PE-transpose E -> E.T, U_A = E.T.T @ q_bf scaled
    by 1/Z, write G blocks (including c*h once the AllReduce lands).

All matmul operands are fp16 (1 cycle/row on PE, like bf16, but 4x finer
rounding); accumulation is fp32 in PSUM; stats are fp32.
"""

import numpy as np

import concourse.bass as bass
import concourse.mybir as mybir
import concourse.tile as tile
from concourse import bacc
from concourse.bass_utils import run_bass_kernel_spmd
from concourse.masks import make_identity

F32 = mybir.dt.float32
# fp16 (10-bit mantissa) runs matmuls at the same 1 cycle/row as bf16 but
# with 4x finer rounding; E = exp(S) <= e^6 stays far below fp16 max.
BF16 = mybir.dt.float16
AF = mybir.ActivationFunctionType

T = 4096
D = 512
NCORES = 8
TL = T // NCORES          # 512 local context rows per core
P = 128
NIB = TL // P             # 4 i-blocks of 128 rows
NJT = T // P              # 32 j-tiles of 128
NJS = T // 512            # 8 j-slabs of 512
NDC = D // P              # 4 d-chunks of 128


def build_kernel(collective=True, compile=True):
    nc = bacc.Bacc("TRN2", target_bir_lowering=False, debug=False,
                   num_devices=NCORES if collective else 1)

    c_dram = nc.dram_tensor("c", [TL, D], F32, kind="ExternalInput").ap()
    q_dram = nc.dram_tensor("q", [T, D], F32, kind="ExternalInput").ap()
    w2p_dram = nc.dram_tensor("w2p", [P, NDC], F32, kind="ExternalInput").ap()
    w3p_dram = nc.dram_tensor("w3p", [P, NDC], F32, kind="ExternalInput").ap()
    g_dram = nc.dram_tensor("g", [TL, 4 * D], F32, kind="ExternalOutput").ap()

    with tile.TileContext(nc) as tc:
        _emit(nc, tc, c_dram, q_dram, w2p_dram, w3p_dram, g_dram,
              collective=collective)

    if compile:
        nc.compile()
    return nc


def _emit(nc, tc, c_dram, q_dram, w2p_dram, w3p_dram, g_dram,
          collective=True):
    from contextlib import ExitStack
    ctx = ExitStack()
    consts = ctx.enter_context(tc.tile_pool(name="consts", bufs=1))
    epool = ctx.enter_context(tc.tile_pool(name="epool", bufs=1))
    etpool = ctx.enter_context(tc.tile_pool(name="etpool", bufs=2))
    spool = ctx.enter_context(tc.tile_pool(name="spool", bufs=2, space="PSUM"))
    tppool = ctx.enter_context(tc.tile_pool(name="tppool", bufs=5, space="PSUM"))
    uapool = ctx.enter_context(tc.tile_pool(name="uapool", bufs=1, space="PSUM"))
    stat = ctx.enter_context(tc.tile_pool(name="stat", bufs=4))
    gout = ctx.enter_context(tc.tile_pool(name="gout", bufs=3))
    dram = ctx.enter_context(tc.tile_pool(name="dram", bufs=1, space="DRAM"))

    # ---- prologue ---------------------------------------------------------
    # ident first: it is tiny gpsimd work but gates every PE transpose, and
    # the gpsimd (Pool) queue also generates all SWDGE cast-DMA descriptors.
    q_bf = consts.tile([P, NJS, 4, D], BF16)  # [p, js, k, d] ; jt = 4*js+k
    # c_bf cast-DMA descriptor first: its transfer overlaps ident setup and
    # it gates PE's first work (the cw3T transposes)
    c_bf = consts.tile([P, NIB, D], BF16)  # [p, ib, d]
    nc.gpsimd.dma_start(out=c_bf,
                        in_=c_dram.rearrange("(ib p) d -> p ib d", p=P))
    ident = consts.tile([P, P], BF16)
    make_identity(nc, ident)
    # dummy exp: pull the ~2.7us ACT table load for exp_and_others into the
    # startup DMA-wait window instead of stalling the first real exp
    warm = consts.tile([1, 1], F32)
    nc.vector.memset(warm, 0.0)
    nc.scalar.activation(out=warm, in_=warm, func=AF.Exp)

    w2p = consts.tile([P, NDC], F32)
    nc.sync.dma_start(out=w2p, in_=w2p_dram)
    w3p = consts.tile([P, NDC], F32)
    nc.sync.dma_start(out=w3p, in_=w3p_dram)

    qT = []  # qT[dc]: (128 d, 4096 j) bf16
    for dc in range(NDC):
        qT.append(consts.tile([P, T], BF16, tag=f"qT{dc}", name=f"qT{dc}"))

    def emit_slab_transposes(js):
        for dc in range(NDC):
            ps = tppool.tile([P, 512], BF16, tag="tp", name=f"tq{js}{dc}")
            for k in range(4):
                nc.tensor.transpose(ps[:, k * P:(k + 1) * P],
                                    q_bf[:, js, k, dc * P:(dc + 1) * P],
                                    ident)
            nc.vector.tensor_copy(out=qT[dc][:, js * 512:(js + 1) * 512],
                                  in_=ps)

    # ---- context: load f32 ------------------------------------------------
    c_nat = []
    for ib in range(NIB):
        t = consts.tile([P, D], F32, tag=f"c_nat{ib}", name=f"c_nat{ib}")
        nc.sync.dma_start(out=t, in_=c_dram[ib * P:(ib + 1) * P, :])
        c_nat.append(t)

    # cw3T[dc] = (context * w3).T chunk PLUS the w2 bias row-constant:
    # W[dc][k,i] = c[i, dc*128+k]*w3[dc*128+k] + w2[dc*128+k].  Because
    #   sum_dc sum_k w2[dc*128+k] * qT[dc][k,j] = (q @ w2)[j] = q2[j],
    # the S matmul then produces  S = (c*w3) @ q.T + q2  directly — the q2
    # bias costs zero extra matmuls (folded into the stationary operand).
    cw3T = []
    for dc in range(NDC):
        ps = tppool.tile([P, TL], BF16, tag="tp")
        for ib in range(NIB):
            nc.tensor.transpose(ps[:, ib * P:(ib + 1) * P],
                                c_bf[:, ib, dc * P:(dc + 1) * P], ident)
        t = consts.tile([P, TL], BF16, tag=f"cw3T{dc}", name=f"cw3T{dc}")
        nc.scalar.activation(out=t, in_=ps, func=AF.Identity,
                             bias=w2p[:, dc:dc + 1],
                             scale=w3p[:, dc:dc + 1])
        cw3T.append(t)

    # ---- persistent per-i-block E, Z-partial and max-partial buffers -----
    e_sb = []
    zpart = []
    mpart = []
    for ib in range(NIB):
        e_sb.append(epool.tile([P, T], BF16, tag=f"e{ib}", name=f"e{ib}"))
        zpart.append(stat.tile([P, NJS], F32, tag=f"zp{ib}", name=f"zp{ib}"))
        mpart.append(stat.tile([P, NJS], F32, tag=f"mp{ib}", name=f"mp{ib}"))

    # ---- phase 1: per j-slab pipeline ------------------------------------
    for js in range(NJS):
        # cast-load one 512-row slab of question as bf16
        nc.gpsimd.dma_start(
            out=q_bf[:, js],
            in_=q_dram[js * 512:(js + 1) * 512, :]
                .rearrange("(k p) d -> p k d", p=P))
        emit_slab_transposes(js)
        # S (with the q2 bias already folded into cw3T) and E per i-block
        for ib in range(NIB):
            ps = spool.tile([P, 512], F32, tag="s")
            for dc in range(NDC):
                nc.tensor.matmul(ps, lhsT=cw3T[dc][:, ib * P:(ib + 1) * P],
                                 rhs=qT[dc][:, js * 512:(js + 1) * 512],
                                 start=(dc == 0), stop=(dc == NDC - 1))
            nc.scalar.activation(out=e_sb[ib][:, js * 512:(js + 1) * 512],
                                 in_=ps, func=AF.Exp,
                                 accum_out=zpart[ib][:, js:js + 1])
            nc.vector.tensor_reduce(out=mpart[ib][:, js:js + 1],
                                    in_=e_sb[ib][:, js * 512:(js + 1) * 512],
                                    axis=mybir.AxisListType.X,
                                    op=mybir.AluOpType.max)

    # ---- phase 2a: per i-block stats + h partial, launch AllReduce -------
    h_ps = spool.tile([P, NDC], F32, tag="s", name="h_ps")  # takes a freed
    # phase-1 S slot; S psums are all drained by the time phase 2a starts
    zinvs = []
    for ib in range(NIB):
        z = stat.tile([P, 1], F32, tag="z")
        nc.vector.tensor_reduce(out=z, in_=zpart[ib],
                                axis=mybir.AxisListType.X,
                                op=mybir.AluOpType.add)
        zinv = stat.tile([P, 1], F32, tag=f"zinv{ib}", name=f"zinv{ib}")
        nc.vector.reciprocal(out=zinv, in_=z)
        zinvs.append(zinv)
        maxe = stat.tile([P, 1], F32, tag="maxe")
        nc.vector.tensor_reduce(out=maxe, in_=mpart[ib],
                                axis=mybir.AxisListType.X,
                                op=mybir.AluOpType.max)
        b = stat.tile([P, 1], F32, tag="b")
        nc.vector.tensor_mul(out=b, in0=maxe, in1=zinv)
        b_bf = stat.tile([P, 1], BF16, tag="b_bf")
        nc.vector.tensor_copy(out=b_bf, in_=b)

        # h partial: h[dc] += c_bf[:, ib, dc].T @ b
        # NOTE start=True clears has_written for the WHOLE bank, so only the
        # very first matmul touching this bank may set it.
        for dc in range(NDC):
            nc.tensor.matmul(h_ps[:, dc:dc + 1],
                             lhsT=c_bf[:, ib, dc * P:(dc + 1) * P],
                             rhs=b_bf,
                             start=(ib == 0 and dc == 0),
                             stop=(ib == NIB - 1 and dc == NDC - 1),
                             skip_group_check=True)

    # h AllReduce launches here; it overlaps the U_A phase below.
    h_sb = stat.tile([P, NDC], F32, tag="h_sb")
    nc.scalar.activation(out=h_sb, in_=h_ps, func=AF.Copy)
    hp_dram = dram.tile([D], F32)
    hs_dram = dram.tile([D], F32)
    hp_ap = hp_dram[:]
    nc.sync.dma_start(out=hp_ap.rearrange("(dc p) -> p dc", p=P), in_=h_sb)
    if collective:
        nc.gpsimd.collective_compute(
            "AllReduce", mybir.AluOpType.add,
            replica_groups=[list(range(NCORES))],
            ins=[hp_dram.opt()], outs=[hs_dram.opt()],
        )
    else:
        nc.sync.dma_start(out=hs_dram[:], in_=hp_dram[:])
    hs_ap = hs_dram[:]
    h_bc = consts.tile([P, D], F32)
    nc.sync.dma_start(
        out=h_bc,
        in_=bass.AP(tensor=hs_ap.tensor, offset=hs_ap.offset,
                    ap=[[0, P], [1, D]]),
    )

    # ---- phase 2b: per i-block E.T, U_A, G -------------------------------
    for ib in range(NIB):
        # G block 0 does not depend on anything but the c load
        nc.sync.dma_start(out=g_dram[ib * P:(ib + 1) * P, 0:D], in_=c_nat[ib])

        # E.T via PE transposes; copies on DVE; U_A matmuls follow per group
        et_sb = etpool.tile([P, T], BF16, tag="et")
        ua_ps = uapool.tile([P, D], F32, tag="ua")
        for jg in range(NJS):
            ps = tppool.tile([P, 512], BF16, tag="tp")
            for k in range(4):
                jt = jg * 4 + k
                nc.tensor.transpose(ps[:, k * P:(k + 1) * P],
                                    e_sb[ib][:, jt * P:(jt + 1) * P], ident)
            nc.vector.tensor_copy(out=et_sb[:, jg * 512:(jg + 1) * 512],
                                  in_=ps)
            for k in range(4):
                jc = jg * 4 + k
                nc.tensor.matmul(ua_ps,
                                 lhsT=et_sb[:, jc * P:(jc + 1) * P],
                                 rhs=q_bf[:, jg, k, :],
                                 start=(jc == 0), stop=(jc == NJT - 1))
        ua = gout.tile([P, D], F32, tag="ua_sb")
        nc.scalar.activation(out=ua, in_=ua_ps, func=AF.Copy, scale=zinvs[ib])

        # G blocks 1..2
        nc.sync.dma_start(out=g_dram[ib * P:(ib + 1) * P, D:2 * D], in_=ua)
        cu = gout.tile([P, D], F32, tag="cu")
        nc.vector.tensor_mul(out=cu, in0=c_nat[ib], in1=ua)
        nc.sync.dma_start(out=g_dram[ib * P:(ib + 1) * P, 2 * D:3 * D], in_=cu)

        # G block 3 (c*h) — h_bc arrives while U_A runs
        ch = gout.tile([P, D], F32, tag="ch")
        nc.vector.tensor_mul(out=ch, in0=c_nat[ib], in1=h_bc)
        nc.sync.dma_start(out=g_dram[ib * P:(ib + 1) * P, 3 * D:4 * D], in_=ch)

    ctx.close()


_NC_CACHE = {}


def _get_nc():
    if "nc" not in _NC_CACHE:
        _NC_CACHE["nc"] = build_kernel()
    return _NC_CACHE["nc"]


def kernel(x: np.ndarray, kernel: np.ndarray) -> np.ndarray:
    nc = _get_nc()

    context = np.ascontiguousarray(x[0, 0]).astype(np.float32)   # (T, D)
    question = np.ascontiguousarray(x[1, 0]).astype(np.float32)  # (T, D)
    w = np.asarray(kernel, dtype=np.float32)
    w2 = w[D:2 * D]
    w3 = w[2 * D:3 * D]
    # partition-major chunk layout: wp[p, dc] = w[dc*128 + p]
    w2p = np.ascontiguousarray(w2.reshape(NDC, P).T)
    w3p = np.ascontiguousarray(w3.reshape(NDC, P).T)

    in_maps = []
    for core in range(NCORES):
        in_maps.append({
            "c": np.ascontiguousarray(context[core * TL:(core + 1) * TL]),
            "q": question,
            "w2p": w2p,
            "w3p": w3p,
        })

    res = run_bass_kernel_spmd(nc, in_maps, core_ids=list(range(NCORES)))
    g = np.concatenate([res.results[core]["g"] for core in range(NCORES)],
                       axis=0)
    return g.astype(np.float32)


# revision 52
# speedup vs baseline: 1.1848x; 1.0101x over previous
"""BiAttention (BiDAF-style) kernel for Trainium2, 8 NeuronCores.

Reference math (T=4096, d=512):
    context  = x[0,0]; question = x[1,0]
    S[i,j]   = w1.c_i + w2.q_j + (c_i*w3).q_j
    A        = softmax_j(S)          # w1.c_i is constant per row -> cancels
    U_A      = A @ question
    b        = max_j A[i,j]          # == max_j E[i,j] / Z_i  with E=exp(S)
    h        = b @ context           # global over T -> one AllReduce
    G        = [context, U_A, context*U_A, context*h]

Sharding: context rows (and rows of S/A/U_A/G) split across 8 cores
(512 rows each); question replicated; h all-reduced (2 KB).

Per-core schedule:
  phase 1 (per 512-wide j-slab): SWDGE cast-load q slab (fp16), PE-transpose
    to qT, then S = W.T @ qT for all four i-blocks where the stationary
    W[dc] = (c*w3).T[dc] + w2[dc] carries the q2 bias for free (because
    sum_dc sum_k w2[k]*qT[dc][k,j] = q2[j]); exp on ACT with fused row-sum
    (Z) accumulation and per-slab row-max partials on DVE.
  phase 2a (per i-block): 1/Z, row-max of E -> b, h-partial matmul into one
    PSUM bank; then the 2 KB h AllReduce launches (hidden under phase 2b).
  phase 2b (per i-block): PE-transpose E -> E.T, U_A = E.T.T @ q_bf scaled
    by 1/Z, write G blocks (including c*h once the AllReduce lands).

All matmul operands are fp16 (1 cycle/row on PE, like bf16, but 4x finer
rounding); accumulation is fp32 in PSUM; stats are fp32.
"""

import numpy as np

import concourse.bass as bass
import concourse.mybir as mybir
import concourse.tile as tile
from concourse import bacc
from concourse.bass_utils import run_bass_kernel_spmd
from concourse.masks import make_identity

F32 = mybir.dt.float32
# fp16 (10-bit mantissa) runs matmuls at the same 1 cycle/row as bf16 but
# with 4x finer rounding; E = exp(S) <= e^6 stays far below fp16 max.
BF16 = mybir.dt.float16
AF = mybir.ActivationFunctionType

T = 4096
D = 512
NCORES = 8
TL = T // NCORES          # 512 local context rows per core
P = 128
NIB = TL // P             # 4 i-blocks of 128 rows
NJT = T // P              # 32 j-tiles of 128
NJS = T // 512            # 8 j-slabs of 512
NDC = D // P              # 4 d-chunks of 128


def build_kernel(collective=True, compile=True):
    nc = bacc.Bacc("TRN2", target_bir_lowering=False, debug=False,
                   num_devices=NCORES if collective else 1)

    c_dram = nc.dram_tensor("c", [TL, D], F32, kind="ExternalInput").ap()
    q_dram = nc.dram_tensor("q", [T, D], F32, kind="ExternalInput").ap()
    w2p_dram = nc.dram_tensor("w2p", [P, NDC], F32, kind="ExternalInput").ap()
    w3p_dram = nc.dram_tensor("w3p", [P, NDC], F32, kind="ExternalInput").ap()
    g_dram = nc.dram_tensor("g", [TL, 4 * D], F32, kind="ExternalOutput").ap()

    with tile.TileContext(nc) as tc:
        _emit(nc, tc, c_dram, q_dram, w2p_dram, w3p_dram, g_dram,
              collective=collective)

    if compile:
        nc.compile()
    return nc


def _emit(nc, tc, c_dram, q_dram, w2p_dram, w3p_dram, g_dram,
          collective=True):
    from contextlib import ExitStack
    ctx = ExitStack()
    consts = ctx.enter_context(tc.tile_pool(name="consts", bufs=1))
    epool = ctx.enter_context(tc.tile_pool(name="epool", bufs=1))
    etpool = ctx.enter_context(tc.tile_pool(name="etpool", bufs=2))
    spool = ctx.enter_context(tc.tile_pool(name="spool", bufs=2, space="PSUM"))
    tppool = ctx.enter_context(tc.tile_pool(name="tppool", bufs=5, space="PSUM"))
    uapool = ctx.enter_context(tc.tile_pool(name="uapool", bufs=1, space="PSUM"))
    stat = ctx.enter_context(tc.tile_pool(name="stat", bufs=4))
    gout = ctx.enter_context(tc.tile_pool(name="gout", bufs=3))
    dram = ctx.enter_context(tc.tile_pool(name="dram", bufs=1, space="DRAM"))

    # ---- prologue ---------------------------------------------------------
    # ident first: it is tiny gpsimd work but gates every PE transpose, and
    # the gpsimd (Pool) queue also generates all SWDGE cast-DMA descriptors.
    q_bf = consts.tile([P, NJS, 4, D], BF16)  # [p, js, k, d] ; jt = 4*js+k
    # c_bf cast-DMA descriptor first: its transfer overlaps ident setup and
    # it gates PE's first work (the cw3T transposes)
    c_bf = consts.tile([P, NIB, D], BF16)  # [p, ib, d]
    nc.gpsimd.dma_start(out=c_bf,
                        in_=c_dram.rearrange("(ib p) d -> p ib d", p=P))
    ident = consts.tile([P, P], BF16)
    make_identity(nc, ident)
    # dummy exp: pull the ~2.7us ACT table load for exp_and_others into the
    # startup DMA-wait window instead of stalling the first real exp
    warm = consts.tile([1, 1], F32)
    nc.vector.memset(warm, 0.0)
    nc.scalar.activation(out=warm, in_=warm, func=AF.Exp)
    # HAM warm-up: dummy matmuls fill the otherwise-idle cold-start DMA wait
    # and bring the PE clock to 2.4 GHz before the real pipeline begins
    wa = consts.tile([P, P], BF16)
    nc.vector.memset(wa, 0.0)
    wb = consts.tile([P, 512], BF16)
    nc.vector.memset(wb, 0.0)
    for wi in range(3):
        wps = tppool.tile([P, 512], F32, tag="tp", name=f"wps{wi}")
        nc.tensor.matmul(wps, lhsT=wa, rhs=wb, start=True, stop=True)

    w2p = consts.tile([P, NDC], F32)
    nc.sync.dma_start(out=w2p, in_=w2p_dram)
    w3p = consts.tile([P, NDC], F32)
    nc.sync.dma_start(out=w3p, in_=w3p_dram)

    qT = []  # qT[dc]: (128 d, 4096 j) bf16
    for dc in range(NDC):
        qT.append(consts.tile([P, T], BF16, tag=f"qT{dc}", name=f"qT{dc}"))

    def emit_slab_transposes(js):
        for dc in range(NDC):
            ps = tppool.tile([P, 512], BF16, tag="tp", name=f"tq{js}{dc}")
            for k in range(4):
                nc.tensor.transpose(ps[:, k * P:(k + 1) * P],
                                    q_bf[:, js, k, dc * P:(dc + 1) * P],
                                    ident)
            nc.vector.tensor_copy(out=qT[dc][:, js * 512:(js + 1) * 512],
                                  in_=ps)

    # ---- context: load f32 ------------------------------------------------
    c_nat = []
    for ib in range(NIB):
        t = consts.tile([P, D], F32, tag=f"c_nat{ib}", name=f"c_nat{ib}")
        nc.sync.dma_start(out=t, in_=c_dram[ib * P:(ib + 1) * P, :])
        c_nat.append(t)

    # cw3T[dc] = (context * w3).T chunk PLUS the w2 bias row-constant:
    # W[dc][k,i] = c[i, dc*128+k]*w3[dc*128+k] + w2[dc*128+k].  Because
    #   sum_dc sum_k w2[dc*128+k] * qT[dc][k,j] = (q @ w2)[j] = q2[j],
    # the S matmul then produces  S = (c*w3) @ q.T + q2  directly — the q2
    # bias costs zero extra matmuls (folded into the stationary operand).
    cw3T = []
    for dc in range(NDC):
        ps = tppool.tile([P, TL], BF16, tag="tp")
        for ib in range(NIB):
            nc.tensor.transpose(ps[:, ib * P:(ib + 1) * P],
                                c_bf[:, ib, dc * P:(dc + 1) * P], ident)
        t = consts.tile([P, TL], BF16, tag=f"cw3T{dc}", name=f"cw3T{dc}")
        nc.scalar.activation(out=t, in_=ps, func=AF.Identity,
                             bias=w2p[:, dc:dc + 1],
                             scale=w3p[:, dc:dc + 1])
        cw3T.append(t)

    # ---- persistent per-i-block E, Z-partial and max-partial buffers -----
    e_sb = []
    zpart = []
    mpart = []
    for ib in range(NIB):
        e_sb.append(epool.tile([P, T], BF16, tag=f"e{ib}", name=f"e{ib}"))
        zpart.append(stat.tile([P, NJS], F32, tag=f"zp{ib}", name=f"zp{ib}"))
        mpart.append(stat.tile([P, NJS], F32, tag=f"mp{ib}", name=f"mp{ib}"))

    # ---- phase 1: per j-slab pipeline ------------------------------------
    for js in range(NJS):
        # cast-load one 512-row slab of question as bf16
        nc.gpsimd.dma_start(
            out=q_bf[:, js],
            in_=q_dram[js * 512:(js + 1) * 512, :]
                .rearrange("(k p) d -> p k d", p=P))
        emit_slab_transposes(js)
        # S (with the q2 bias already folded into cw3T) and E per i-block
        for ib in range(NIB):
            ps = spool.tile([P, 512], F32, tag="s")
            for dc in range(NDC):
                nc.tensor.matmul(ps, lhsT=cw3T[dc][:, ib * P:(ib + 1) * P],
                                 rhs=qT[dc][:, js * 512:(js + 1) * 512],
                                 start=(dc == 0), stop=(dc == NDC - 1))
            nc.scalar.activation(out=e_sb[ib][:, js * 512:(js + 1) * 512],
                                 in_=ps, func=AF.Exp,
                                 accum_out=zpart[ib][:, js:js + 1])
            nc.vector.tensor_reduce(out=mpart[ib][:, js:js + 1],
                                    in_=e_sb[ib][:, js * 512:(js + 1) * 512],
                                    axis=mybir.AxisListType.X,
                                    op=mybir.AluOpType.max)

    # ---- phase 2a: per i-block stats + h partial, launch AllReduce -------
    h_ps = spool.tile([P, NDC], F32, tag="s", name="h_ps")  # takes a freed
    # phase-1 S slot; S psums are all drained by the time phase 2a starts
    zinvs = []
    for ib in range(NIB):
        z = stat.tile([P, 1], F32, tag="z")
        nc.vector.tensor_reduce(out=z, in_=zpart[ib],
                                axis=mybir.AxisListType.X,
                                op=mybir.AluOpType.add)
        zinv = stat.tile([P, 1], F32, tag=f"zinv{ib}", name=f"zinv{ib}")
        nc.vector.reciprocal(out=zinv, in_=z)
        zinvs.append(zinv)
        maxe = stat.tile([P, 1], F32, tag="maxe")
        nc.vector.tensor_reduce(out=maxe, in_=mpart[ib],
                                axis=mybir.AxisListType.X,
                                op=mybir.AluOpType.max)
        b = stat.tile([P, 1], F32, tag="b")
        nc.vector.tensor_mul(out=b, in0=maxe, in1=zinv)
        b_bf = stat.tile([P, 1], BF16, tag="b_bf")
        nc.vector.tensor_copy(out=b_bf, in_=b)

        # h partial: h[dc] += c_bf[:, ib, dc].T @ b
        # NOTE start=True clears has_written for the WHOLE bank, so only the
        # very first matmul touching this bank may set it.
        for dc in range(NDC):
            nc.tensor.matmul(h_ps[:, dc:dc + 1],
                             lhsT=c_bf[:, ib, dc * P:(dc + 1) * P],
                             rhs=b_bf,
                             start=(ib == 0 and dc == 0),
                             stop=(ib == NIB - 1 and dc == NDC - 1),
                             skip_group_check=True)

    # h AllReduce launches here; it overlaps the U_A phase below.
    h_sb = stat.tile([P, NDC], F32, tag="h_sb")
    nc.scalar.activation(out=h_sb, in_=h_ps, func=AF.Copy)
    hp_dram = dram.tile([D], F32)
    hs_dram = dram.tile([D], F32)
    hp_ap = hp_dram[:]
    nc.sync.dma_start(out=hp_ap.rearrange("(dc p) -> p dc", p=P), in_=h_sb)
    if collective:
        nc.gpsimd.collective_compute(
            "AllReduce", mybir.AluOpType.add,
            replica_groups=[list(range(NCORES))],
            ins=[hp_dram.opt()], outs=[hs_dram.opt()],
        )
    else:
        nc.sync.dma_start(out=hs_dram[:], in_=hp_dram[:])
    hs_ap = hs_dram[:]
    h_bc = consts.tile([P, D], F32)
    nc.sync.dma_start(
        out=h_bc,
        in_=bass.AP(tensor=hs_ap.tensor, offset=hs_ap.offset,
                    ap=[[0, P], [1, D]]),
    )

    # ---- phase 2b: per i-block E.T, U_A, G -------------------------------
    for ib in range(NIB):
        # G block 0 does not depend on anything but the c load
        nc.sync.dma_start(out=g_dram[ib * P:(ib + 1) * P, 0:D], in_=c_nat[ib])

        # E.T via PE transposes; copies on DVE; U_A matmuls follow per group
        et_sb = etpool.tile([P, T], BF16, tag="et")
        ua_ps = uapool.tile([P, D], F32, tag="ua")
        for jg in range(NJS):
            ps = tppool.tile([P, 512], BF16, tag="tp")
            for k in range(4):
                jt = jg * 4 + k
                nc.tensor.transpose(ps[:, k * P:(k + 1) * P],
                                    e_sb[ib][:, jt * P:(jt + 1) * P], ident)
            nc.vector.tensor_copy(out=et_sb[:, jg * 512:(jg + 1) * 512],
                                  in_=ps)
            for k in range(4):
                jc = jg * 4 + k
                nc.tensor.matmul(ua_ps,
                                 lhsT=et_sb[:, jc * P:(jc + 1) * P],
                                 rhs=q_bf[:, jg, k, :],
                                 start=(jc == 0), stop=(jc == NJT - 1))
        ua = gout.tile([P, D], F32, tag="ua_sb")
        nc.scalar.activation(out=ua, in_=ua_ps, func=AF.Copy, scale=zinvs[ib])

        # G blocks 1..2
        nc.sync.dma_start(out=g_dram[ib * P:(ib + 1) * P, D:2 * D], in_=ua)
        cu = gout.tile([P, D], F32, tag="cu")
        nc.vector.tensor_mul(out=cu, in0=c_nat[ib], in1=ua)
        nc.sync.dma_start(out=g_dram[ib * P:(ib + 1) * P, 2 * D:3 * D], in_=cu)

        # G block 3 (c*h) — h_bc arrives while U_A runs
        ch = gout.tile([P, D], F32, tag="ch")
        nc.vector.tensor_mul(out=ch, in0=c_nat[ib], in1=h_bc)
        nc.sync.dma_start(out=g_dram[ib * P:(ib + 1) * P, 3 * D:4 * D], in_=ch)

    ctx.close()


_NC_CACHE = {}


def _get_nc():
    if "nc" not in _NC_CACHE:
        _NC_CACHE["nc"] = build_kernel()
    return _NC_CACHE["nc"]


def kernel(x: np.ndarray, kernel: np.ndarray) -> np.ndarray:
    nc = _get_nc()

    context = np.ascontiguousarray(x[0, 0]).astype(np.float32)   # (T, D)
    question = np.ascontiguousarray(x[1, 0]).astype(np.float32)  # (T, D)
    w = np.asarray(kernel, dtype=np.float32)
    w2 = w[D:2 * D]
    w3 = w[2 * D:3 * D]
    # partition-major chunk layout: wp[p, dc] = w[dc*128 + p]
    w2p = np.ascontiguousarray(w2.reshape(NDC, P).T)
    w3p = np.ascontiguousarray(w3.reshape(NDC, P).T)

    in_maps = []
    for core in range(NCORES):
        in_maps.append({
            "c": np.ascontiguousarray(context[core * TL:(core + 1) * TL]),
            "q": question,
            "w2p": w2p,
            "w3p": w3p,
        })

    res = run_bass_kernel_spmd(nc, in_maps, core_ids=list(range(NCORES)))
    g = np.concatenate([res.results[core]["g"] for core in range(NCORES)],
                       axis=0)
    return g.astype(np.float32)
